# revision 10
# baseline (speedup 1.0000x reference)
"""Trainium2 Bass kernel for a 3-layer GNN message-passing block.

Reference computation (per layer i):
    x1 = h @ Wfc[i] + bfc[i]                        # [N_SUB, D]
    x2 = scatter_mean(h, idx) @ Wsum[i] + bsum[i]   # [NUM_GRAPHS, D]
    h  = elu(x1 + x2[idx])
then
    out = relu(scatter_mean(h, idx) @ Wf1 + bf1) @ Wf2 + bf2

Strategy: data-parallel over 8 NeuronCores; the sorted graph index lets us
split graphs contiguously; each core owns 5 windows of <=128 graphs / <=2560
subgraph rows (rows padded + permuted host-side so every window is exactly 20
chunks of 128 rows). scatter_mean and the x2[idx] gather are one-hot matmuls
on the TensorEngine; biases fold in via K=1 matmuls; 1/count folds into the
ACT copy that reads the scatter PSUM.

Precision: the fc contraction is split — input dims 0..255 run as one fp8
(e4m3) DoubleRow matmul (2 virtual k-tiles per pass, ~2x), dims 256..511 stay
fp16.  The scatter/gather/x2/head path stays fp16, and the host folds the fp8
weight-quantization residual E = Wfc[:256] - fp8(Wfc[:256]) into Wsum, which
cancels the per-graph common-mode of the fp8 error (the part the final
graph-mean would amplify).  ELU is computed as max(z, min(exp(z),1)-1):
one ACT op + two DVE ops per chunk.
"""

import numpy as np
import ml_dtypes

P = 128
D = 512
N_SUB = 100000
NUM_GRAPHS = 4096
N_LAYERS = 3
NUM_TASKS = 10
N_CORES = 8
WIN_PER_CORE = 5
CH_PER_WIN = 20
ROWS_PER_WIN = CH_PER_WIN * P            # 2560
N_LOC = WIN_PER_CORE * ROWS_PER_WIN      # 12800 padded rows per core
CHUNKS = N_LOC // P                      # 100
G_WIN = P                                # graph slots per window
G_LOC = WIN_PER_CORE * G_WIN             # 640 graph slots per core
N_WIN_TOTAL = N_CORES * WIN_PER_CORE     # 40
DBLK = D // P                            # 4
D2 = 2 * D                               # 1024
D2BLK = D2 // P
PREFETCH = 6     # next-layer hT transposes prefetched (= stream pool bufs)
K8 = 256         # input dims 0..K8-1 go through the fp8 DoubleRow path
KHI = D - K8     # fp16 remainder of the contraction
NBHI = KHI // P  # fp16 k-blocks per fc matmul

_cached = {}


def _f16():
    return np.float16


def _q8(x):
    """Round-trip through TRN-compatible e4m3 (clip to +-240)."""
    return np.clip(np.asarray(x, np.float32), -240.0, 240.0).astype(
        ml_dtypes.float8_e4m3fn)


# ----------------------------------------------------------------- host prep

def _pack_windows(counts):
    """Split graphs 0..NUM_GRAPHS-1 into N_WIN_TOTAL contiguous windows with
    <= G_WIN graphs and <= ROWS_PER_WIN rows each, roughly row-balanced."""
    total = int(counts.sum())
    target = total / N_WIN_TOTAL
    wins = []
    g = 0
    rows_done = 0
    for w in range(N_WIN_TOTAL):
        g0 = g
        rows_w = 0
        while g < NUM_GRAPHS:
            c = int(counts[g])
            if rows_w + c > ROWS_PER_WIN or (g - g0) >= G_WIN:
                break
            if (w < N_WIN_TOTAL - 1 and rows_w > 0
                    and rows_done + rows_w + c > (w + 1) * target):
                remaining = total - (rows_done + rows_w)
                if remaining <= (N_WIN_TOTAL - w - 1) * ROWS_PER_WIN * 0.98:
                    break
            rows_w += c
            g += 1
        while g < NUM_GRAPHS and counts[g] == 0 and (g - g0) < G_WIN:
            g += 1
        rows_done += rows_w
        wins.append((g0, g))
    assert g == NUM_GRAPHS, f"window packing failed: {g}/{NUM_GRAPHS}"
    return wins


def _build_core_inputs(h, idx, counts, starts, wins, core, shared):
    f16 = _f16()
    h_pad = np.zeros((N_LOC, D), dtype=np.float32)
    slot = np.full(N_LOC, -1, dtype=np.int64)
    invc = np.zeros((P, WIN_PER_CORE), dtype=np.float32)  # [g_in_win, w]
    gmap = []
    for lw in range(WIN_PER_CORE):
        g0, g1 = wins[core * WIN_PER_CORE + lw]
        r0, r1 = int(starts[g0]), int(starts[g1])
        n = r1 - r0
        h_pad[lw * ROWS_PER_WIN: lw * ROWS_PER_WIN + n] = h[r0:r1]
        slot[lw * ROWS_PER_WIN: lw * ROWS_PER_WIN + n] = \
            lw * G_WIN + (idx[r0:r1] - g0)
        for j, g in enumerate(range(g0, g1)):
            invc[j, lw] = 1.0 / max(int(counts[g]), 1)
            gmap.append((g, lw * G_WIN + j))
    wsc = np.zeros((CHUNKS, P, P), dtype=np.float32)
    for c in range(CHUNKS):
        w = c // CH_PER_WIN
        s = slot[c * P:(c + 1) * P]
        real = np.nonzero(s >= 0)[0]
        wsc[c][real, s[real] - w * G_WIN] = 1.0
    wga = np.transpose(wsc, (0, 2, 1))
    # sbuf layout [P, CHUNKS*P]: partition p holds chunk-c block at cols 128c..
    wsc_flat = np.ascontiguousarray(
        np.transpose(wsc, (1, 0, 2)).reshape(P, CHUNKS * P)).astype(f16)
    wga_flat = np.ascontiguousarray(
        np.transpose(wga, (1, 0, 2)).reshape(P, CHUNKS * P)).astype(f16)
    h3 = h_pad.reshape(CHUNKS, P, DBLK, P)
    h16t = h3.transpose(3, 0, 2, 1).reshape(P, CHUNKS, D)     # [p, c, (b r)]
    # layer-0 fc operands from the host: fp8 low half + fp16 high half
    h16t8 = np.ascontiguousarray(h16t[:, :, :K8].reshape(P, CHUNKS * K8))
    h16thi = np.ascontiguousarray(h16t[:, :, K8:].reshape(P, CHUNKS * KHI))
    # layer-0 scatter_mean + x2 computed host-side (depends only on inputs)
    Wsum0, cbias0 = shared["_wsum0"], shared["_cbias0"]
    h16 = h_pad.astype(f16).astype(np.float32)
    ssum = np.zeros((G_LOC, D), dtype=np.float32)
    np.add.at(ssum, slot[slot >= 0], h16[slot >= 0])
    mean0 = ssum * invc.T.reshape(G_LOC, 1)
    x2w0 = (mean0 @ Wsum0 + cbias0).astype(f16)      # [G_LOC, D]
    x2w0b = np.ascontiguousarray(
        x2w0.reshape(WIN_PER_CORE, G_WIN, D).transpose(1, 0, 2)
        .reshape(G_WIN, WIN_PER_CORE * D))
    in_map = {
        "h16t8": _q8(h16t8.astype(np.float32)),
        "h16thi": h16thi.astype(f16),
        "x2w0b": x2w0b,
        "wsc": wsc_flat,
        "wga": wga_flat,
        "invc": invc,
        **{k: v for k, v in shared.items() if not k.startswith("_")},
    }
    return in_map, gmap


def _prep_shared(Wfc, bfc, Wsum, bsum, Wf1, bf1, Wf2, bf2):
    f16 = _f16()
    # fp8 low half of Wfc, packed for DoubleRow rhs: [p, j(2), dout] per layer
    wfc8 = np.zeros((P, N_LAYERS, 2, D), dtype=np.float32)
    E = np.zeros((N_LAYERS, K8, D), dtype=np.float32)
    for l in range(N_LAYERS):
        W8 = _q8(Wfc[l][:K8])
        E[l] = Wfc[l][:K8] - W8.astype(np.float32)
        wfc8[:, l] = W8.astype(np.float32).reshape(2, P, D).transpose(1, 0, 2)
    wfc8_flat = _q8(np.ascontiguousarray(wfc8.reshape(P, N_LAYERS * 2 * D)))
    # fp16 high half of Wfc: blocks b=2,3 -> [P, l, b', D]
    wfc16 = np.ascontiguousarray(
        Wfc[:, K8:].reshape(N_LAYERS, NBHI, P, D).transpose(2, 0, 1, 3)
        .reshape(P, N_LAYERS * NBHI * D)).astype(f16)
    # Wsum' = Wsum + E (cancels the graph-mean of the fp8 fc error)
    Wsum_eff = Wsum.astype(np.float32).copy()
    Wsum_eff[:, :K8] += E
    wsum = np.ascontiguousarray(
        Wsum_eff.reshape(N_LAYERS, DBLK, P, D).transpose(2, 0, 1, 3)
        .reshape(P, N_LAYERS * DBLK * D)).astype(f16)
    cbias = np.ascontiguousarray((bfc + bsum).reshape(1, N_LAYERS * D)).astype(f16)
    # wf1 rhs blocks [b][half] = Wf1[b*P:(b+1)*P, half*D:(half+1)*D]
    wf1 = np.ascontiguousarray(
        Wf1.reshape(DBLK, P, 2, D).transpose(1, 0, 2, 3)
        .reshape(P, DBLK * D2)).astype(f16)
    bf1w = np.ascontiguousarray(bf1.reshape(1, D2)).astype(f16)
    # wf2 blocks [q] = Wf2[q*P:(q+1)*P, :]; packed [P, 8*NUM_TASKS]
    wf2 = np.ascontiguousarray(
        Wf2.reshape(D2BLK, P, NUM_TASKS).transpose(1, 0, 2)
        .reshape(P, D2BLK * NUM_TASKS)).astype(f16)
    bf2w = np.ascontiguousarray(bf2.reshape(1, NUM_TASKS)).astype(f16)
    return {
        "wfc8": wfc8_flat, "wfc16": wfc16, "wsum": wsum, "cbias": cbias,
        "wf1": wf1, "bf1w": bf1w, "wf2": wf2, "bf2w": bf2w,
        "_wsum0": Wsum_eff[0],
        "_cbias0": (bfc[0] + bsum[0]).astype(np.float32)[None, :],
    }


# -------------------------------------------------------------- bass program

def _build_program():
    from contextlib import ExitStack

    import concourse.mybir as mybir
    import concourse.tile as tile
    from concourse import bacc

    f16 = mybir.dt.float16
    fp8 = mybir.dt.float8e4
    f32 = mybir.dt.float32
    AF = mybir.ActivationFunctionType
    ALU = mybir.AluOpType
    DR = mybir.MatmulPerfMode.DoubleRow

    nc = bacc.Bacc("TRN2", debug=False, target_bir_lowering=False,
                   num_devices=N_CORES, dynamic_dma_scratch_size=2048)

    h16t8_d = nc.dram_tensor("h16t8", [P, CHUNKS * K8], fp8, kind="ExternalInput")
    h16thi_d = nc.dram_tensor("h16thi", [P, CHUNKS * KHI], f16, kind="ExternalInput")
    x2w0_d = nc.dram_tensor("x2w0b", [G_WIN, WIN_PER_CORE * D], f16,
                            kind="ExternalInput")
    wsc_d = nc.dram_tensor("wsc", [P, CHUNKS * P], f16, kind="ExternalInput")
    wga_d = nc.dram_tensor("wga", [P, CHUNKS * P], f16, kind="ExternalInput")
    invc_d = nc.dram_tensor("invc", [P, WIN_PER_CORE], f32, kind="ExternalInput")
    wfc8_d = nc.dram_tensor("wfc8", [P, N_LAYERS * 2 * D], fp8, kind="ExternalInput")
    wfc16_d = nc.dram_tensor("wfc16", [P, N_LAYERS * NBHI * D], f16,
                             kind="ExternalInput")
    wsum_d = nc.dram_tensor("wsum", [P, N_LAYERS * DBLK * D], f16, kind="ExternalInput")
    cbias_d = nc.dram_tensor("cbias", [1, N_LAYERS * D], f16, kind="ExternalInput")
    wf1_d = nc.dram_tensor("wf1", [P, DBLK * D2BLK * P], f16, kind="ExternalInput")
    bf1_d = nc.dram_tensor("bf1w", [1, D2], f16, kind="ExternalInput")
    wf2_d = nc.dram_tensor("wf2", [P, D2BLK * NUM_TASKS], f16, kind="ExternalInput")
    bf2_d = nc.dram_tensor("bf2w", [1, NUM_TASKS], f16, kind="ExternalInput")
    out_d = nc.dram_tensor("out", [NUM_TASKS, G_LOC], f32, kind="ExternalOutput")

    with tile.TileContext(nc) as tc, ExitStack() as ctx:
        const = ctx.enter_context(tc.tile_pool(name="const", bufs=1))
        hpool = ctx.enter_context(tc.tile_pool(name="h", bufs=1))
        stream = ctx.enter_context(tc.tile_pool(name="stream", bufs=6))
        work = ctx.enter_context(tc.tile_pool(name="work", bufs=2))
        x2pool = ctx.enter_context(tc.tile_pool(name="x2", bufs=1))
        psum = ctx.enter_context(tc.tile_pool(name="psum", bufs=2, space="PSUM"))
        psx1 = ctx.enter_context(tc.tile_pool(name="psx1", bufs=4, space="PSUM"))

        ones = const.tile([1, P], f16, tag="ones")
        nc.vector.memset(ones[:], 1.0)

        # ---- constants + layer-0 stream.  First-chunk critical path goes
        # first, split across the sync and scalar (ACT) HWDGE queues.
        wfc8b = const.tile([P, N_LAYERS * 2 * D], fp8, tag="wfc8b")
        nc.scalar.dma_start(wfc8b[:, :2 * D], wfc8_d[:, :2 * D])
        wfc16b = const.tile([P, N_LAYERS * NBHI * D], f16, tag="wfc16b")
        nc.scalar.dma_start(wfc16b[:, :NBHI * D], wfc16_d[:, :NBHI * D])

        # layer-0 fc operands stream in [p, c, k] layout; alternate queues
        hT8_l0 = []
        hThi_l0 = []

        def load_hT0(c):
            eng = nc.sync if c % 2 == 0 else nc.scalar
            t8 = stream.tile([P, K8], fp8, tag="hT8", name=f"hT08_{c}", bufs=6)
            eng.dma_start(t8[:], h16t8_d[:, c * K8:(c + 1) * K8])
            hT8_l0.append(t8)
            th = stream.tile([P, KHI], f16, tag="hThi", name=f"hT0h_{c}", bufs=4)
            eng.dma_start(th[:], h16thi_d[:, c * KHI:(c + 1) * KHI])
            hThi_l0.append(th)

        load_hT0(0)
        x2w0_t = []
        for w in range(WIN_PER_CORE):
            t0w = x2pool.tile([P, D], f16, tag=f"x2w{w}", name=f"x2w0_{w}")
            (nc.sync if w == 0 else nc.scalar).dma_start(
                t0w[:], x2w0_d[:, w * D:(w + 1) * D])
            x2w0_t.append(t0w)
        WCH = CH_PER_WIN * P
        wgab_w = [const.tile([P, WCH], f16, tag=f"wgab{k}", name=f"wgab{k}")
                  for k in range(WIN_PER_CORE)]
        wscb_w = [const.tile([P, WCH], f16, tag=f"wscb{k}", name=f"wscb{k}")
                  for k in range(WIN_PER_CORE)]
        invc_t = const.tile([P, WIN_PER_CORE], f32, tag="invc")
        nc.sync.dma_start(invc_t[:], invc_d[:, :])
        h_t = [hpool.tile([P, D], f16, tag=f"h{c}", name=f"h{c}")
               for c in range(CHUNKS)]
        wsumb = const.tile([P, N_LAYERS * DBLK * D], f16, tag="wsumb")
        cbiasb = const.tile([1, N_LAYERS * D], f16, tag="cbiasb")
        # one-hot tensors stream in small pieces, interleaved with the layer-0
        # fc operand stream in need-order so neither queue gets a long blob
        PIECES = 8
        PW = WCH // PIECES
        LAG0 = 4
        onehot_q = []      # (need_chunk, tile, dram, col0, col1)
        for k in range(WIN_PER_CORE):
            for j in range(PIECES):
                c0, c1 = k * WCH + j * PW, k * WCH + (j + 1) * PW
                need_ga = k * CH_PER_WIN + j * CH_PER_WIN // PIECES
                onehot_q.append((need_ga, wgab_w[k], wga_d, j * PW,
                                 (j + 1) * PW, c0, c1))
                onehot_q.append((need_ga + LAG0, wscb_w[k], wsc_d, j * PW,
                                 (j + 1) * PW, c0, c1))
        onehot_q.sort(key=lambda t: t[0])
        qi = 0

        def drain_onehot(upto):
            nonlocal qi
            while qi < len(onehot_q) and onehot_q[qi][0] <= upto:
                _, tl, dr, a0, a1, c0, c1 = onehot_q[qi]
                (nc.sync if qi % 2 == 0 else nc.scalar).dma_start(
                    tl[:, a0:a1], dr[:, c0:c1])
                qi += 1

        drain_onehot(3)                     # chunk-0..3 gather/scatter pieces
        for c in range(1, CHUNKS):
            load_hT0(c)
            drain_onehot(c + 4)
            if c == 30:
                # layer-1 Wsum' slice for the layer-0 window tails (~50us in);
                # the layer-0 slice is never read (x2w0 comes from the host)
                nc.scalar.dma_start(wsumb[:, DBLK * D:2 * DBLK * D],
                                    wsum_d[:, DBLK * D:2 * DBLK * D])
                nc.scalar.dma_start(cbiasb[:], cbias_d[:, :])
        drain_onehot(10 ** 9)
        nc.scalar.dma_start(wsumb[:, 2 * DBLK * D:], wsum_d[:, 2 * DBLK * D:])
        nc.scalar.dma_start(wfc8b[:, 2 * D:], wfc8_d[:, 2 * D:])
        nc.scalar.dma_start(wfc16b[:, NBHI * D:], wfc16_d[:, NBHI * D:])
        wf1b = const.tile([P, DBLK * D2BLK * P], f16, tag="wf1b")
        nc.scalar.dma_start(wf1b[:], wf1_d[:, :])
        bf1_t = const.tile([1, D2], f16, tag="bf1")
        nc.scalar.dma_start(bf1_t[:], bf1_d[:, :])
        wf2b = const.tile([P, D2BLK * NUM_TASKS], f16, tag="wf2b")
        nc.scalar.dma_start(wf2b[:], wf2_d[:, :])
        bf2_t = const.tile([1, NUM_TASKS], f16, tag="bf2")
        nc.scalar.dma_start(bf2_t[:], bf2_d[:, :])

        def wfc8_s(layer):
            return wfc8b[:, layer * 2 * D:(layer + 1) * 2 * D].rearrange(
                "p (j n) -> p j n", j=2)

        def wfc16_s(layer, b):
            i = layer * NBHI + b
            return wfc16b[:, i * D:(i + 1) * D]

        def wsum_s(layer, b):
            return wsumb[:, (layer * DBLK + b) * D:(layer * DBLK + b + 1) * D]

        def cbias_s(layer):
            return cbiasb[:, layer * D:(layer + 1) * D]

        def wf1_h(b, half):
            i = b * 2 + half
            return wf1b[:, i * D:(i + 1) * D]

        def wf2_s(q):
            return wf2b[:, q * NUM_TASKS:(q + 1) * NUM_TASKS]

        def wsc_c(c):
            return wscb_w[c // CH_PER_WIN][:, (c % CH_PER_WIN) * P:
                                           (c % CH_PER_WIN + 1) * P]

        def wga_c(c):
            return wgab_w[c // CH_PER_WIN][:, (c % CH_PER_WIN) * P:
                                           (c % CH_PER_WIN + 1) * P]

        def x2_window(meanT, w, layer):
            """x2 = meanT.T @ Wsum' + (bfc+bsum), as f16 [g, d]."""
            ps = psum.tile([P, D], f32, tag="x2")
            for b in range(DBLK):
                nc.tensor.matmul(ps[:], lhsT=meanT[:, b * P:(b + 1) * P],
                                 rhs=wsum_s(layer, b),
                                 start=(b == 0), stop=False)
            nc.tensor.matmul(ps[:], lhsT=ones[:, :P], rhs=cbias_s(layer),
                             start=False, stop=True)
            x2w = x2pool.tile([P, D], f16, tag=f"x2w{w}", name=f"x2w{w}")
            nc.scalar.activation(x2w[:], ps[:], AF.Copy)
            return x2w[:]

        # layer-0 x2 comes precomputed from the host
        x2ws = {w: x2w0_t[w][:] for w in range(WIN_PER_CORE)}

        out_sb = const.tile([NUM_TASKS, G_LOC], f32, tag="out")

        head_pend = {}

        def head_t(w, meanT):
            """t = relu(hg @ Wf1 + bf1), transposed; out-MMs deferred."""
            t = work.tile([P, D2], f16, tag="tT", bufs=1, name=f"t{w}")
            tTh = []
            for half in range(2):
                ps = psx1.tile([P, D], f32, tag="x1", name=f"hps{w}_{half}")
                for b in range(DBLK):
                    nc.tensor.matmul(ps[:], lhsT=meanT[:, b * P:(b + 1) * P],
                                     rhs=wf1_h(b, half),
                                     start=(b == 0), stop=False)
                nc.tensor.matmul(ps[:], lhsT=ones[:, :P],
                                 rhs=bf1_t[:, half * D:(half + 1) * D],
                                 start=False, stop=True)
                nc.scalar.activation(t[:, half * D:(half + 1) * D],
                                     ps[:], AF.Relu)
                th = work.tile([P, D], f16, tag=f"tTh{half}", bufs=1,
                               name=f"tTh{w}_{half}")
                nc.sync.dma_start(th[:].rearrange("p (b r) -> p b r", b=DBLK),
                                  t[:, half * D:(half + 1) * D],
                                  transpose=True)
                tTh.append(th)
            head_pend[w] = tTh

        def head_out(w):
            tTh = head_pend.pop(w)
            pso = psum.tile([NUM_TASKS, P], f32, tag="x2", name=f"pso{w}")
            for q in range(D2BLK):
                nc.tensor.matmul(pso[:], lhsT=wf2_s(q),
                                 rhs=tTh[q // 4][:, (q % 4) * P:
                                                 (q % 4 + 1) * P],
                                 start=(q == 0), stop=False)
            nc.tensor.matmul(pso[:], lhsT=bf2_t[:], rhs=ones[:, :P],
                             start=False, stop=True)
            nc.vector.tensor_copy(out_sb[:, w * P:(w + 1) * P], pso[:])

        def head_window(w, meanT):
            head_t(w, meanT)

        # ---- update passes; each folds the NEXT context's scatter (layer
        # l+1's, or the head's) in with a small lag so the PE never has a
        # serial scatter phase after layer 0.
        LAG = 4
        hT8_next, hThi_next, hTfull_next = hT8_l0, hThi_l0, []
        for layer in range(N_LAYERS):
            hT8s, hThis, hTfulls = hT8_next, hThi_next, hTfull_next
            if layer > 0:
                for c in range(len(hTfulls), CHUNKS):
                    hTt = stream.tile([P, D], f16, tag="hT", name=f"hT{c}",
                                      bufs=7)
                    nc.sync.dma_start(hTt[:].rearrange("p (b r) -> p b r",
                                                       b=DBLK),
                                      h_t[c][:], transpose=True)
                    hTfulls.append(hTt)
                    t8 = stream.tile([P, K8], fp8, tag="hT8", name=f"hT8_{c}",
                                     bufs=6)
                    nc.vector.tensor_copy(t8[:], hTt[:, :K8])
                    hT8s.append(t8)
            hT8_next, hThi_next, hTfull_next = [], [], []
            nxt_x2ws = {}
            sc_state = {}
            pend = {}

            def emit_next_scatter(c, layer=layer, sc_state=sc_state,
                                  pend=pend):
                i = c % CH_PER_WIN
                w = c // CH_PER_WIN
                if i == 0:
                    sc_state["ps"] = psum.tile([P, D], f32, tag="sc",
                                               name=f"sc{layer}_{w}")
                nc.tensor.matmul(sc_state["ps"][:], lhsT=wsc_c(c),
                                 rhs=h_t[c][:],
                                 start=(i == 0), stop=(i == CH_PER_WIN - 1))
                if i == CH_PER_WIN - 1:
                    mean = work.tile([P, D], f16, tag="mean", bufs=1,
                                     name=f"mean{layer}_{w}")
                    nc.scalar.activation(mean[:], sc_state["ps"][:], AF.Copy,
                                         scale=invc_t[:, w:w + 1])
                    meanT = work.tile([P, D], f16, tag="meanT", bufs=1,
                                      name=f"meanT{layer}_{w}")
                    nc.sync.dma_start(
                        meanT[:].rearrange("p (b r) -> p b r", b=DBLK),
                        mean[:], transpose=True)
                    pend[w] = meanT

            def emit_window_tail(w, layer=layer, nxt_x2ws=nxt_x2ws,
                                 pend=pend):
                meanT = pend.pop(w)
                if layer < N_LAYERS - 1:
                    nxt_x2ws[w] = x2_window(meanT, w, layer + 1)
                else:
                    head_window(w, meanT)

            for c in range(CHUNKS):
                w = c // CH_PER_WIN
                ps = psx1.tile([P, D], f32, tag="x1")
                # fp8 DoubleRow pass over input dims 0..K8-1 (2 k-tiles)
                nc.tensor.matmul(ps[:],
                                 lhsT=hT8s[c][:].rearrange(
                                     "p (j m) -> p j m", j=2),
                                 rhs=wfc8_s(layer),
                                 start=True, stop=False, perf_mode=DR,
                                 skip_group_check=True)
                # fp16 passes over dims K8..D-1
                if layer == 0:
                    hi = hThis[c]
                    hi_s = lambda b: hi[:, b * P:(b + 1) * P]
                else:
                    full = hTfulls[c]
                    hi_s = lambda b: full[:, K8 + b * P:K8 + (b + 1) * P]
                for b in range(NBHI):
                    nc.tensor.matmul(ps[:], lhsT=hi_s(b),
                                     rhs=wfc16_s(layer, b),
                                     start=False, stop=False,
                                     skip_group_check=True)
                nc.tensor.matmul(ps[:], lhsT=wga_c(c), rhs=x2ws[w],
                                 start=False, stop=True, skip_group_check=True)
                # ELU: h = max(z, min(exp(z), 1) - 1)
                e = work.tile([P, D], f16, tag="e")
                nc.scalar.activation(e[:], ps[:], AF.Exp)
                me = work.tile([P, D], f16, tag="me")
                nc.vector.tensor_scalar(me[:], e[:], 1.0, -1.0,
                                        op0=ALU.min, op1=ALU.add)
                nc.vector.tensor_tensor(h_t[c][:], ps[:], me[:], op=ALU.max)
                if layer < N_LAYERS - 1 and c < PREFETCH:
                    nx = stream.tile([P, D], f16, tag="hTp", name=f"hTp{c}")
                    nc.sync.dma_start(
                        nx[:].rearrange("p (b r) -> p b r", b=DBLK),
                        h_t[c][:], transpose=True)
                    hTfull_next.append(nx)
                    nx8 = stream.tile([P, K8], fp8, tag="hT8p",
                                      name=f"hT8p{c}")
                    nc.vector.tensor_copy(nx8[:], nx[:, :K8])
                    hT8_next.append(nx8)
                if c >= LAG:
                    emit_next_scatter(c - LAG)
                cw = (c - LAG - 8) // CH_PER_WIN     # window whose meanT has
                if c >= LAG + 8 and (c - LAG - 8) % CH_PER_WIN == CH_PER_WIN - 1:
                    emit_window_tail(cw)             # had 8 chunks to settle
                cw2 = (c - LAG - 14) // CH_PER_WIN
                if (c >= LAG + 14
                        and (c - LAG - 14) % CH_PER_WIN == CH_PER_WIN - 1
                        and cw2 in head_pend):
                    head_out(cw2)
            for c in range(CHUNKS - LAG, CHUNKS):
                emit_next_scatter(c)
            for w in sorted(pend):
                emit_window_tail(w)
            for w in sorted(head_pend):
                head_out(w)
            x2ws = nxt_x2ws

        nc.sync.dma_start(out_d[:, :], out_sb[:])

    nc.compile()
    return nc


# ------------------------------------------------------------------- kernel

def kernel(**inputs):
    h = np.asarray(inputs["h_subgraph"], dtype=np.float32)
    idx = np.asarray(inputs["subgraph_idx_batch"]).astype(np.int64)
    if not np.all(idx[:-1] <= idx[1:]):        # defensive: index must be sorted
        order = np.argsort(idx, kind="stable")
        h, idx = h[order], idx[order]

    counts = np.bincount(idx, minlength=NUM_GRAPHS)
    starts = np.concatenate([[0], np.cumsum(counts)])
    wins = _pack_windows(counts)
    shared = _prep_shared(
        np.asarray(inputs["Wfc"], np.float32), np.asarray(inputs["bfc"], np.float32),
        np.asarray(inputs["Wsum"], np.float32), np.asarray(inputs["bsum"], np.float32),
        np.asarray(inputs["Wf1"], np.float32), np.asarray(inputs["bf1"], np.float32),
        np.asarray(inputs["Wf2"], np.float32), np.asarray(inputs["bf2"], np.float32),
    )

    in_maps = []
    gmaps = []
    for core in range(N_CORES):
        m, gm = _build_core_inputs(h, idx, counts, starts, wins, core, shared)
        in_maps.append(m)
        gmaps.append(gm)

    _cached["in_maps"] = in_maps
    if "nc" not in _cached:
        _cached["nc"] = _build_program()
    nc = _cached["nc"]

    from concourse import bass_utils
    res = bass_utils.run_bass_kernel_spmd(
        nc, in_maps, core_ids=list(range(N_CORES)))

    out = np.zeros((NUM_GRAPHS, NUM_TASKS), dtype=np.float32)
    for core in range(N_CORES):
        o = res.results[core]["out"]           # [10, 640]
        for g, s in gmaps[core]:
            out[g] = o[:, s]
    return out


# revision 17
# speedup vs baseline: 1.0647x; 1.0647x over previous
"""Trainium2 Bass kernel for a 3-layer GNN message-passing block.

Reference computation (per layer i):
    x1 = h @ Wfc[i] + bfc[i]                        # [N_SUB, D]
    x2 = scatter_mean(h, idx) @ Wsum[i] + bsum[i]   # [NUM_GRAPHS, D]
    h  = elu(x1 + x2[idx])
then
    out = relu(scatter_mean(h, idx) @ Wf1 + bf1) @ Wf2 + bf2

Strategy: data-parallel over 8 NeuronCores; the sorted graph index lets us
split graphs contiguously; each core owns 5 windows of <=128 graphs / <=2560
subgraph rows (rows padded + permuted host-side so every window is exactly 20
chunks of 128 rows). scatter_mean and the x2[idx] gather are one-hot matmuls
on the TensorEngine; biases fold in via K=1 matmuls; 1/count folds into the
ACT copy that reads the scatter PSUM.

Precision: the fc contraction is split — input dims 0..255 run as one fp8
(e4m3) DoubleRow matmul (2 virtual k-tiles per pass, ~2x), dims 256..511 stay
fp16.  The scatter/gather/x2/head path stays fp16, and the host folds the fp8
weight-quantization residual E = Wfc[:256] - fp8(Wfc[:256]) into Wsum, which
cancels the per-graph common-mode of the fp8 error (the part the final
graph-mean would amplify).  ELU is computed as max(z, min(exp(z),1)-1):
one ACT op + two DVE ops per chunk.
"""

import numpy as np
import ml_dtypes

P = 128
D = 512
N_SUB = 100000
NUM_GRAPHS = 4096
N_LAYERS = 3
NUM_TASKS = 10
N_CORES = 8
WIN_PER_CORE = 5
CH_PER_WIN = 20
ROWS_PER_WIN = CH_PER_WIN * P            # 2560
N_LOC = WIN_PER_CORE * ROWS_PER_WIN      # 12800 padded rows per core
CHUNKS = N_LOC // P                      # 100
G_WIN = P                                # graph slots per window
G_LOC = WIN_PER_CORE * G_WIN             # 640 graph slots per core
N_WIN_TOTAL = N_CORES * WIN_PER_CORE     # 40
DBLK = D // P                            # 4
D2 = 2 * D                               # 1024
D2BLK = D2 // P
PREFETCH = 6     # next-layer hT transposes prefetched (= stream pool bufs)
K8 = 256         # input dims 0..K8-1 go through the fp8 DoubleRow path
KHI = D - K8     # fp16 remainder of the contraction
NBHI = KHI // P  # fp16 k-blocks per fc matmul

_cached = {}


def _f16():
    return np.float16


def _q8(x):
    """Round-trip through TRN-compatible e4m3 (clip to +-240)."""
    return np.clip(np.asarray(x, np.float32), -240.0, 240.0).astype(
        ml_dtypes.float8_e4m3fn)


# ----------------------------------------------------------------- host prep

def _pack_windows(counts):
    """Split graphs 0..NUM_GRAPHS-1 into N_WIN_TOTAL contiguous windows with
    <= G_WIN graphs and <= ROWS_PER_WIN rows each, roughly row-balanced."""
    total = int(counts.sum())
    target = total / N_WIN_TOTAL
    wins = []
    g = 0
    rows_done = 0
    for w in range(N_WIN_TOTAL):
        g0 = g
        rows_w = 0
        while g < NUM_GRAPHS:
            c = int(counts[g])
            if rows_w + c > ROWS_PER_WIN or (g - g0) >= G_WIN:
                break
            if (w < N_WIN_TOTAL - 1 and rows_w > 0
                    and rows_done + rows_w + c > (w + 1) * target):
                remaining = total - (rows_done + rows_w)
                if remaining <= (N_WIN_TOTAL - w - 1) * ROWS_PER_WIN * 0.98:
                    break
            rows_w += c
            g += 1
        while g < NUM_GRAPHS and counts[g] == 0 and (g - g0) < G_WIN:
            g += 1
        rows_done += rows_w
        wins.append((g0, g))
    assert g == NUM_GRAPHS, f"window packing failed: {g}/{NUM_GRAPHS}"
    return wins


def _build_core_inputs(h, idx, counts, starts, wins, core, shared):
    f16 = _f16()
    h_pad = np.zeros((N_LOC, D), dtype=np.float32)
    slot = np.full(N_LOC, -1, dtype=np.int64)
    invc = np.zeros((P, WIN_PER_CORE), dtype=np.float32)  # [g_in_win, w]
    gmap = []
    for lw in range(WIN_PER_CORE):
        g0, g1 = wins[core * WIN_PER_CORE + lw]
        r0, r1 = int(starts[g0]), int(starts[g1])
        n = r1 - r0
        h_pad[lw * ROWS_PER_WIN: lw * ROWS_PER_WIN + n] = h[r0:r1]
        slot[lw * ROWS_PER_WIN: lw * ROWS_PER_WIN + n] = \
            lw * G_WIN + (idx[r0:r1] - g0)
        for j, g in enumerate(range(g0, g1)):
            invc[j, lw] = 1.0 / max(int(counts[g]), 1)
            gmap.append((g, lw * G_WIN + j))
    wsc = np.zeros((CHUNKS, P, P), dtype=np.float32)
    for c in range(CHUNKS):
        w = c // CH_PER_WIN
        s = slot[c * P:(c + 1) * P]
        real = np.nonzero(s >= 0)[0]
        wsc[c][real, s[real] - w * G_WIN] = 1.0
    wga = np.transpose(wsc, (0, 2, 1))
    # sbuf layout [P, CHUNKS*P]: partition p holds chunk-c block at cols 128c..
    wsc_flat = np.ascontiguousarray(
        np.transpose(wsc, (1, 0, 2)).reshape(P, CHUNKS * P)).astype(f16)
    wga_flat = np.ascontiguousarray(
        np.transpose(wga, (1, 0, 2)).reshape(P, CHUNKS * P)).astype(f16)
    h3 = h_pad.reshape(CHUNKS, P, DBLK, P)
    h16t = h3.transpose(3, 0, 2, 1).reshape(P, CHUNKS, D)     # [p, c, (b r)]
    # layer-0 fc operands from the host: fp8 low half + fp16 high half
    h16t8 = np.ascontiguousarray(h16t[:, :, :K8].reshape(P, CHUNKS * K8))
    h16thi = np.ascontiguousarray(h16t[:, :, K8:].reshape(P, CHUNKS * KHI))
    # layer-0 scatter_mean + x2 computed host-side (depends only on inputs)
    Wsum0, cbias0 = shared["_wsum0"], shared["_cbias0"]
    h16 = h_pad.astype(f16).astype(np.float32)
    ssum = np.zeros((G_LOC, D), dtype=np.float32)
    np.add.at(ssum, slot[slot >= 0], h16[slot >= 0])
    mean0 = ssum * invc.T.reshape(G_LOC, 1)
    x2w0 = (mean0 @ Wsum0 + cbias0).astype(f16)      # [G_LOC, D]
    x2w0b = np.ascontiguousarray(
        x2w0.reshape(WIN_PER_CORE, G_WIN, D).transpose(1, 0, 2)
        .reshape(G_WIN, WIN_PER_CORE * D))
    in_map = {
        "h16t8": _q8(h16t8.astype(np.float32)),
        "h16thi": h16thi.astype(f16),
        "x2w0b": x2w0b,
        "wsc": wsc_flat,
        "wga": wga_flat,
        "invc": invc,
        **{k: v for k, v in shared.items() if not k.startswith("_")},
    }
    return in_map, gmap


def _prep_shared(Wfc, bfc, Wsum, bsum, Wf1, bf1, Wf2, bf2):
    f16 = _f16()
    # fp8 low half of Wfc, packed for DoubleRow rhs: [p, j(2), dout] per layer
    wfc8 = np.zeros((P, N_LAYERS, 2, D), dtype=np.float32)
    E = np.zeros((N_LAYERS, K8, D), dtype=np.float32)
    for l in range(N_LAYERS):
        W8 = _q8(Wfc[l][:K8])
        E[l] = Wfc[l][:K8] - W8.astype(np.float32)
        wfc8[:, l] = W8.astype(np.float32).reshape(2, P, D).transpose(1, 0, 2)
    wfc8_flat = _q8(np.ascontiguousarray(wfc8.reshape(P, N_LAYERS * 2 * D)))
    # fp16 high half of Wfc: blocks b=2,3 -> [P, l, b', D]
    wfc16 = np.ascontiguousarray(
        Wfc[:, K8:].reshape(N_LAYERS, NBHI, P, D).transpose(2, 0, 1, 3)
        .reshape(P, N_LAYERS * NBHI * D)).astype(f16)
    # Wsum' = Wsum + E (cancels the graph-mean of the fp8 fc error)
    Wsum_eff = Wsum.astype(np.float32).copy()
    Wsum_eff[:, :K8] += E
    wsum = np.ascontiguousarray(
        Wsum_eff.reshape(N_LAYERS, DBLK, P, D).transpose(2, 0, 1, 3)
        .reshape(P, N_LAYERS * DBLK * D)).astype(f16)
    cbias = np.ascontiguousarray((bfc + bsum).reshape(1, N_LAYERS * D)).astype(f16)
    # wf1 rhs blocks [b][half] = Wf1[b*P:(b+1)*P, half*D:(half+1)*D]
    wf1 = np.ascontiguousarray(
        Wf1.reshape(DBLK, P, 2, D).transpose(1, 0, 2, 3)
        .reshape(P, DBLK * D2)).astype(f16)
    bf1w = np.ascontiguousarray(bf1.reshape(1, D2)).astype(f16)
    # wf2 blocks [q] = Wf2[q*P:(q+1)*P, :]; packed [P, 8*NUM_TASKS]
    wf2 = np.ascontiguousarray(
        Wf2.reshape(D2BLK, P, NUM_TASKS).transpose(1, 0, 2)
        .reshape(P, D2BLK * NUM_TASKS)).astype(f16)
    bf2w = np.ascontiguousarray(bf2.reshape(1, NUM_TASKS)).astype(f16)
    return {
        "wfc8": wfc8_flat, "wfc16": wfc16, "wsum": wsum, "cbias": cbias,
        "wf1": wf1, "bf1w": bf1w, "wf2": wf2, "bf2w": bf2w,
        "_wsum0": Wsum_eff[0],
        "_cbias0": (bfc[0] + bsum[0]).astype(np.float32)[None, :],
    }


# -------------------------------------------------------------- bass program

def _build_program():
    from contextlib import ExitStack

    import concourse.mybir as mybir
    import concourse.tile as tile
    from concourse import bacc

    f16 = mybir.dt.float16
    fp8 = mybir.dt.float8e4
    f32 = mybir.dt.float32
    AF = mybir.ActivationFunctionType
    ALU = mybir.AluOpType
    DR = mybir.MatmulPerfMode.DoubleRow

    nc = bacc.Bacc("TRN2", debug=False, target_bir_lowering=False,
                   num_devices=N_CORES, dynamic_dma_scratch_size=2048)

    h16t8_d = nc.dram_tensor("h16t8", [P, CHUNKS * K8], fp8, kind="ExternalInput")
    h16thi_d = nc.dram_tensor("h16thi", [P, CHUNKS * KHI], f16, kind="ExternalInput")
    x2w0_d = nc.dram_tensor("x2w0b", [G_WIN, WIN_PER_CORE * D], f16,
                            kind="ExternalInput")
    wsc_d = nc.dram_tensor("wsc", [P, CHUNKS * P], f16, kind="ExternalInput")
    wga_d = nc.dram_tensor("wga", [P, CHUNKS * P], f16, kind="ExternalInput")
    invc_d = nc.dram_tensor("invc", [P, WIN_PER_CORE], f32, kind="ExternalInput")
    wfc8_d = nc.dram_tensor("wfc8", [P, N_LAYERS * 2 * D], fp8, kind="ExternalInput")
    wfc16_d = nc.dram_tensor("wfc16", [P, N_LAYERS * NBHI * D], f16,
                             kind="ExternalInput")
    wsum_d = nc.dram_tensor("wsum", [P, N_LAYERS * DBLK * D], f16, kind="ExternalInput")
    cbias_d = nc.dram_tensor("cbias", [1, N_LAYERS * D], f16, kind="ExternalInput")
    wf1_d = nc.dram_tensor("wf1", [P, DBLK * D2BLK * P], f16, kind="ExternalInput")
    bf1_d = nc.dram_tensor("bf1w", [1, D2], f16, kind="ExternalInput")
    wf2_d = nc.dram_tensor("wf2", [P, D2BLK * NUM_TASKS], f16, kind="ExternalInput")
    bf2_d = nc.dram_tensor("bf2w", [1, NUM_TASKS], f16, kind="ExternalInput")
    out_d = nc.dram_tensor("out", [NUM_TASKS, G_LOC], f32, kind="ExternalOutput")

    with tile.TileContext(nc) as tc, ExitStack() as ctx:
        const = ctx.enter_context(tc.tile_pool(name="const", bufs=1))
        hpool = ctx.enter_context(tc.tile_pool(name="h", bufs=1))
        stream = ctx.enter_context(tc.tile_pool(name="stream", bufs=6))
        work = ctx.enter_context(tc.tile_pool(name="work", bufs=2))
        x2pool = ctx.enter_context(tc.tile_pool(name="x2", bufs=1))
        psum = ctx.enter_context(tc.tile_pool(name="psum", bufs=2, space="PSUM"))
        psx1 = ctx.enter_context(tc.tile_pool(name="psx1", bufs=4, space="PSUM"))

        ones = const.tile([1, P], f16, tag="ones")
        nc.vector.memset(ones[:], 1.0)

        # ---- constants + layer-0 stream.  First-chunk critical path goes
        # first, split across the sync and scalar (ACT) HWDGE queues.
        wfc8b = const.tile([P, N_LAYERS * 2 * D], fp8, tag="wfc8b")
        nc.scalar.dma_start(wfc8b[:, :2 * D], wfc8_d[:, :2 * D])
        wfc16b = const.tile([P, N_LAYERS * NBHI * D], f16, tag="wfc16b")
        nc.scalar.dma_start(wfc16b[:, :NBHI * D], wfc16_d[:, :NBHI * D])

        # layer-0 fc operands stream in [p, c, k] layout; pairs of chunks per
        # DMA (bigger transfers run closer to peak), alternating queues
        hT8_l0 = []
        hThi_l0 = []

        def load_hT0(c):
            # loads chunks c and c+1 in one pair of DMAs
            eng = nc.sync if (c // 2) % 2 == 0 else nc.scalar
            n = min(2, CHUNKS - c)
            t8 = stream.tile([P, n * K8], fp8, tag="hT8" if n == 2 else "hT8x",
                             name=f"hT08_{c}", bufs=3)
            eng.dma_start(t8[:], h16t8_d[:, c * K8:(c + n) * K8])
            th = stream.tile([P, n * KHI], f16,
                             tag="hThi" if n == 2 else "hThix",
                             name=f"hT0h_{c}", bufs=2)
            eng.dma_start(th[:], h16thi_d[:, c * KHI:(c + n) * KHI])
            for i in range(n):
                hT8_l0.append((t8, i * K8))
                hThi_l0.append((th, i * KHI))

        load_hT0(0)
        x2w0_t = []
        for w in range(WIN_PER_CORE):
            t0w = x2pool.tile([P, D], f16, tag=f"x2w{w}", name=f"x2w0_{w}")
            (nc.sync if w == 0 else nc.scalar).dma_start(
                t0w[:], x2w0_d[:, w * D:(w + 1) * D])
            x2w0_t.append(t0w)
        WCH = CH_PER_WIN * P
        wgab_w = [const.tile([P, WCH], f16, tag=f"wgab{k}", name=f"wgab{k}")
                  for k in range(WIN_PER_CORE)]
        wscb_w = [const.tile([P, WCH], f16, tag=f"wscb{k}", name=f"wscb{k}")
                  for k in range(WIN_PER_CORE)]
        invc_t = const.tile([P, WIN_PER_CORE], f32, tag="invc")
        nc.sync.dma_start(invc_t[:], invc_d[:, :])
        h_t = [hpool.tile([P, D], f16, tag=f"h{c}", name=f"h{c}")
               for c in range(CHUNKS)]
        wsumb = const.tile([P, N_LAYERS * DBLK * D], f16, tag="wsumb")
        cbiasb = const.tile([1, N_LAYERS * D], f16, tag="cbiasb")
        # one-hot tensors stream in small pieces, interleaved with the layer-0
        # fc operand stream in need-order so neither queue gets a long blob
        PIECES = 8
        PW = WCH // PIECES
        LAG0 = 4
        onehot_q = []      # (need_chunk, tile, dram, col0, col1)
        for k in range(WIN_PER_CORE):
            for j in range(PIECES):
                c0, c1 = k * WCH + j * PW, k * WCH + (j + 1) * PW
                need_ga = k * CH_PER_WIN + j * CH_PER_WIN // PIECES
                onehot_q.append((need_ga, wgab_w[k], wga_d, j * PW,
                                 (j + 1) * PW, c0, c1))
                onehot_q.append((need_ga + LAG0, wscb_w[k], wsc_d, j * PW,
                                 (j + 1) * PW, c0, c1))
        onehot_q.sort(key=lambda t: t[0])
        qi = 0

        def drain_onehot(upto):
            nonlocal qi
            while qi < len(onehot_q) and onehot_q[qi][0] <= upto:
                _, tl, dr, a0, a1, c0, c1 = onehot_q[qi]
                (nc.sync if qi % 2 == 0 else nc.scalar).dma_start(
                    tl[:, a0:a1], dr[:, c0:c1])
                qi += 1

        drain_onehot(3)                     # chunk-0..3 gather/scatter pieces
        for c in range(2, CHUNKS, 2):
            load_hT0(c)
            drain_onehot(c + 5)
            if c == 30:
                # layer-1 Wsum' slice for the layer-0 window tails (~50us in);
                # the layer-0 slice is never read (x2w0 comes from the host)
                nc.scalar.dma_start(wsumb[:, DBLK * D:2 * DBLK * D],
                                    wsum_d[:, DBLK * D:2 * DBLK * D])
                nc.scalar.dma_start(cbiasb[:], cbias_d[:, :])
        drain_onehot(10 ** 9)
        nc.scalar.dma_start(wsumb[:, 2 * DBLK * D:], wsum_d[:, 2 * DBLK * D:])
        nc.scalar.dma_start(wfc8b[:, 2 * D:], wfc8_d[:, 2 * D:])
        nc.scalar.dma_start(wfc16b[:, NBHI * D:], wfc16_d[:, NBHI * D:])
        wf1b = const.tile([P, DBLK * D2BLK * P], f16, tag="wf1b")
        nc.scalar.dma_start(wf1b[:], wf1_d[:, :])
        bf1_t = const.tile([1, D2], f16, tag="bf1")
        nc.scalar.dma_start(bf1_t[:], bf1_d[:, :])
        wf2b = const.tile([P, D2BLK * NUM_TASKS], f16, tag="wf2b")
        nc.scalar.dma_start(wf2b[:], wf2_d[:, :])
        bf2_t = const.tile([1, NUM_TASKS], f16, tag="bf2")
        nc.scalar.dma_start(bf2_t[:], bf2_d[:, :])

        def wfc8_s(layer):
            return wfc8b[:, layer * 2 * D:(layer + 1) * 2 * D].rearrange(
                "p (j n) -> p j n", j=2)

        def wfc16_s(layer, b):
            i = layer * NBHI + b
            return wfc16b[:, i * D:(i + 1) * D]

        def wsum_s(layer, b):
            return wsumb[:, (layer * DBLK + b) * D:(layer * DBLK + b + 1) * D]

        def cbias_s(layer):
            return cbiasb[:, layer * D:(layer + 1) * D]

        def wf1_h(b, half):
            i = b * 2 + half
            return wf1b[:, i * D:(i + 1) * D]

        def wf2_s(q):
            return wf2b[:, q * NUM_TASKS:(q + 1) * NUM_TASKS]

        def wsc_c(c):
            return wscb_w[c // CH_PER_WIN][:, (c % CH_PER_WIN) * P:
                                           (c % CH_PER_WIN + 1) * P]

        def wga_c(c):
            return wgab_w[c // CH_PER_WIN][:, (c % CH_PER_WIN) * P:
                                           (c % CH_PER_WIN + 1) * P]

        def x2_window(meanT, w, layer):
            """x2 = meanT.T @ Wsum' + (bfc+bsum), as f16 [g, d]."""
            ps = psum.tile([P, D], f32, tag="x2")
            for b in range(DBLK):
                nc.tensor.matmul(ps[:], lhsT=meanT[:, b * P:(b + 1) * P],
                                 rhs=wsum_s(layer, b),
                                 start=(b == 0), stop=False)
            nc.tensor.matmul(ps[:], lhsT=ones[:, :P], rhs=cbias_s(layer),
                             start=False, stop=True)
            x2w = x2pool.tile([P, D], f16, tag=f"x2w{w}", name=f"x2w{w}")
            nc.scalar.activation(x2w[:], ps[:], AF.Copy)
            return x2w[:]

        # layer-0 x2 comes precomputed from the host
        x2ws = {w: x2w0_t[w][:] for w in range(WIN_PER_CORE)}

        out_sb = const.tile([NUM_TASKS, G_LOC], f32, tag="out")

        head_pend = {}

        def head_t(w, meanT):
            """t = relu(hg @ Wf1 + bf1), transposed; out-MMs deferred."""
            t = work.tile([P, D2], f16, tag="tT", bufs=1, name=f"t{w}")
            tTh = []
            for half in range(2):
                ps = psx1.tile([P, D], f32, tag="x1", name=f"hps{w}_{half}")
                for b in range(DBLK):
                    nc.tensor.matmul(ps[:], lhsT=meanT[:, b * P:(b + 1) * P],
                                     rhs=wf1_h(b, half),
                                     start=(b == 0), stop=False)
                nc.tensor.matmul(ps[:], lhsT=ones[:, :P],
                                 rhs=bf1_t[:, half * D:(half + 1) * D],
                                 start=False, stop=True)
                nc.scalar.activation(t[:, half * D:(half + 1) * D],
                                     ps[:], AF.Relu)
                th = work.tile([P, D], f16, tag=f"tTh{half}", bufs=1,
                               name=f"tTh{w}_{half}")
                nc.sync.dma_start(th[:].rearrange("p (b r) -> p b r", b=DBLK),
                                  t[:, half * D:(half + 1) * D],
                                  transpose=True)
                tTh.append(th)
            head_pend[w] = tTh

        def head_out(w):
            tTh = head_pend.pop(w)
            pso = psum.tile([NUM_TASKS, P], f32, tag="x2", name=f"pso{w}")
            for q in range(D2BLK):
                nc.tensor.matmul(pso[:], lhsT=wf2_s(q),
                                 rhs=tTh[q // 4][:, (q % 4) * P:
                                                 (q % 4 + 1) * P],
                                 start=(q == 0), stop=False)
            nc.tensor.matmul(pso[:], lhsT=bf2_t[:], rhs=ones[:, :P],
                             start=False, stop=True)
            nc.vector.tensor_copy(out_sb[:, w * P:(w + 1) * P], pso[:])

        def head_window(w, meanT):
            head_t(w, meanT)

        # ---- update passes; each folds the NEXT context's scatter (layer
        # l+1's, or the head's) in with a small lag so the PE never has a
        # serial scatter phase after layer 0.
        LAG = 4
        hT8_next, hThi_next, hTfull_next = hT8_l0, hThi_l0, []
        for layer in range(N_LAYERS):
            hT8s, hThis, hTfulls = hT8_next, hThi_next, hTfull_next
            if layer > 0:
                for c in range(len(hTfulls), CHUNKS):
                    hTt = stream.tile([P, D], f16, tag="hT", name=f"hT{c}",
                                      bufs=7)
                    nc.sync.dma_start(hTt[:].rearrange("p (b r) -> p b r",
                                                       b=DBLK),
                                      h_t[c][:], transpose=True)
                    hTfulls.append(hTt)
                    t8 = stream.tile([P, K8], fp8, tag="hT8q", name=f"hT8_{c}",
                                     bufs=6)
                    nc.vector.tensor_copy(t8[:], hTt[:, :K8])
                    hT8s.append((t8, 0))
            hT8_next, hThi_next, hTfull_next = [], [], []
            nxt_x2ws = {}
            sc_state = {}
            pend = {}

            def emit_next_scatter(c, layer=layer, sc_state=sc_state,
                                  pend=pend):
                i = c % CH_PER_WIN
                w = c // CH_PER_WIN
                if i == 0:
                    sc_state["ps"] = psum.tile([P, D], f32, tag="sc",
                                               name=f"sc{layer}_{w}")
                nc.tensor.matmul(sc_state["ps"][:], lhsT=wsc_c(c),
                                 rhs=h_t[c][:],
                                 start=(i == 0), stop=(i == CH_PER_WIN - 1))
                if i == CH_PER_WIN - 1:
                    mean = work.tile([P, D], f16, tag="mean", bufs=1,
                                     name=f"mean{layer}_{w}")
                    nc.scalar.activation(mean[:], sc_state["ps"][:], AF.Copy,
                                         scale=invc_t[:, w:w + 1])
                    meanT = work.tile([P, D], f16, tag="meanT", bufs=1,
                                      name=f"meanT{layer}_{w}")
                    nc.sync.dma_start(
                        meanT[:].rearrange("p (b r) -> p b r", b=DBLK),
                        mean[:], transpose=True)
                    pend[w] = meanT

            def emit_window_tail(w, layer=layer, nxt_x2ws=nxt_x2ws,
                                 pend=pend):
                meanT = pend.pop(w)
                if layer < N_LAYERS - 1:
                    nxt_x2ws[w] = x2_window(meanT, w, layer + 1)
                else:
                    head_window(w, meanT)

            for c in range(CHUNKS):
                w = c // CH_PER_WIN
                ps = psx1.tile([P, D], f32, tag="x1")
                # fp8 DoubleRow pass over input dims 0..K8-1 (2 k-tiles)
                t8c, off8 = hT8s[c]
                nc.tensor.matmul(ps[:],
                                 lhsT=t8c[:, off8:off8 + K8].rearrange(
                                     "p (j m) -> p j m", j=2),
                                 rhs=wfc8_s(layer),
                                 start=True, stop=False, perf_mode=DR,
                                 skip_group_check=True)
                # fp16 passes over dims K8..D-1
                if layer == 0:
                    hi, offh = hThis[c]
                    hi_s = lambda b: hi[:, offh + b * P:offh + (b + 1) * P]
                else:
                    full = hTfulls[c]
                    hi_s = lambda b: full[:, K8 + b * P:K8 + (b + 1) * P]
                for b in range(NBHI):
                    nc.tensor.matmul(ps[:], lhsT=hi_s(b),
                                     rhs=wfc16_s(layer, b),
                                     start=False, stop=False,
                                     skip_group_check=True)
                nc.tensor.matmul(ps[:], lhsT=wga_c(c), rhs=x2ws[w],
                                 start=False, stop=True, skip_group_check=True)
                # ELU: h = max(z, min(exp(z), 1) - 1)
                e = work.tile([P, D], f16, tag="e")
                nc.scalar.activation(e[:], ps[:], AF.Exp)
                me = work.tile([P, D], f16, tag="me")
                nc.vector.tensor_scalar(me[:], e[:], 1.0, -1.0,
                                        op0=ALU.min, op1=ALU.add)
                nc.vector.tensor_tensor(h_t[c][:], ps[:], me[:], op=ALU.max)
                if layer < N_LAYERS - 1 and c < PREFETCH:
                    nx = stream.tile([P, D], f16, tag="hTp", name=f"hTp{c}")
                    nc.sync.dma_start(
                        nx[:].rearrange("p (b r) -> p b r", b=DBLK),
                        h_t[c][:], transpose=True)
                    hTfull_next.append(nx)
                    nx8 = stream.tile([P, K8], fp8, tag="hT8p",
                                      name=f"hT8p{c}")
                    nc.vector.tensor_copy(nx8[:], nx[:, :K8])
                    hT8_next.append((nx8, 0))
                if c >= LAG:
                    emit_next_scatter(c - LAG)
                cw = (c - LAG - 8) // CH_PER_WIN     # window whose meanT has
                if c >= LAG + 8 and (c - LAG - 8) % CH_PER_WIN == CH_PER_WIN - 1:
                    emit_window_tail(cw)             # had 8 chunks to settle
                cw2 = (c - LAG - 14) // CH_PER_WIN
                if (c >= LAG + 14
                        and (c - LAG - 14) % CH_PER_WIN == CH_PER_WIN - 1
                        and cw2 in head_pend):
                    head_out(cw2)
            for c in range(CHUNKS - LAG, CHUNKS):
                emit_next_scatter(c)
            for w in sorted(pend):
                emit_window_tail(w)
            for w in sorted(head_pend):
                head_out(w)
            x2ws = nxt_x2ws

        nc.sync.dma_start(out_d[:, :], out_sb[:])

    nc.compile()
    return nc


# ------------------------------------------------------------------- kernel

def kernel(**inputs):
    h = np.asarray(inputs["h_subgraph"], dtype=np.float32)
    idx = np.asarray(inputs["subgraph_idx_batch"]).astype(np.int64)
    if not np.all(idx[:-1] <= idx[1:]):        # defensive: index must be sorted
        order = np.argsort(idx, kind="stable")
        h, idx = h[order], idx[order]

    counts = np.bincount(idx, minlength=NUM_GRAPHS)
    starts = np.concatenate([[0], np.cumsum(counts)])
    wins = _pack_windows(counts)
    shared = _prep_shared(
        np.asarray(inputs["Wfc"], np.float32), np.asarray(inputs["bfc"], np.float32),
        np.asarray(inputs["Wsum"], np.float32), np.asarray(inputs["bsum"], np.float32),
        np.asarray(inputs["Wf1"], np.float32), np.asarray(inputs["bf1"], np.float32),
        np.asarray(inputs["Wf2"], np.float32), np.asarray(inputs["bf2"], np.float32),
    )

    in_maps = []
    gmaps = []
    for core in range(N_CORES):
        m, gm = _build_core_inputs(h, idx, counts, starts, wins, core, shared)
        in_maps.append(m)
        gmaps.append(gm)

    _cached["in_maps"] = in_maps
    if "nc" not in _cached:
        _cached["nc"] = _build_program()
    nc = _cached["nc"]

    from concourse import bass_utils
    res = bass_utils.run_bass_kernel_spmd(
        nc, in_maps, core_ids=list(range(N_CORES)))

    out = np.zeros((NUM_GRAPHS, NUM_TASKS), dtype=np.float32)
    for core in range(N_CORES):
        o = res.results[core]["out"]           # [10, 640]
        for g, s in gmaps[core]:
            out[g] = o[:, s]
    return out
